# revision 5
# baseline (speedup 1.0000x reference)
"""Trainium2 Bass kernel for nn_AttentionalGNN — SINGLE-CORE version.

Math (identical to the 8-core kernel):
  xs/xt = standardize(p_src/p_tar).T ; ds/dt = mlp_dis(standardize(dis).T)
  delta0 = attn(xs, xt, xt); delta1 = attn(xt, xs, xs)
  ps = delta0*xt; pt = delta1*xs
  delta0' = attn(ds, dt, ps); delta1' = attn(dt, ds, pt)
  out_s = xs + mlp(cat(xs, delta0')); out_t likewise
  return ||mean_n(out_s) - mean_n(out_t)||^2

Why single-core: per-execution host dispatch through the axon tunnel has a
large multi-device gang cost (~1.0-3.7 ms for 2-8 cores, load-dependent)
but ~0-0.4 ms for a single device, and input buffers are device-resident
(bytes are free per-exec). One core running the whole problem (~2.4 ms
device span, ScalarE-exp-bound) beats every multi-core configuration on
the end-to-end per-exec metric, and has far lower variance.

Single-core consequences: no collectives at all; standardize/BN stats are
local (bn_stats/bn_aggr); m1 runs twice (stats pass + relu pass) instead
of materializing the 4 MB hidden activation.

Attention (4096 queries x 4096 keys, transposed scores: keys on
partitions, queries on free): loops qg-outer (8 groups of 512 queries),
h-inner. Per (qg,h): K=64 score matmuls in chunks of 3 key m-tiles ->
one [128,3,512] exp per chunk on ScalarE (scale=1/8, bf16, no max
subtraction) -> PV accumulates [ones|V] one chunk behind; psum row 64 is
the softmax denominator (-> partition 0 via DMA -> fast reciprocal -> PE
broadcast -> DVE normalize, rolling one (qg,h) step behind). After a qg's
4 heads normalize, the per-qg merge runs while the next qg streams, so
the attention-out buffer An is only 2 qg-slots deep. ScalarE does ONLY
exp (~2.2 ms total); every other elementwise op is on DVE.

SBUF is the binding constraint (~224 KB/partition): the 16 KB/partition
slabs are alias-chained by lifetime -- x1fl (mlp_dis hidden) -> gd0
(gated delta0, V of r2a) -> dp1 (r2b out); raw dis -> gd1; d_bf (round-1
delta) -> dp0.

Standardization is never materialized: round-1 attention uses row-scaled
weight copies (Wq' = diag(inv) Wq) on the RAW bf16 inputs with on-device
bias corrections; K bias dropped (softmax-invariant); mlp_dis consumes
raw dis through a row-scaled d1 (shift absorbed by its BN); m1 consumes
raw p through row-scaled weights; mean_n(xs) == 0 collapses the final MLP
to m1 -> BN -> relu -> channel sums -> m2 matvec -> ||.||^2. Head
channels permuted host-side (d*4+h -> h*64+d); V bias folded into the
merge bias.
"""

import numpy as np
import ml_dtypes

D, H, HD, S, N, EPS = 256, 4, 64, 128, 4096, 1e-5
NC = 1
MT = N // 128            # 32 key m-tiles
QG = N // 512            # 8 query groups
HB = HD + 1              # per-head V^T block: [ones | V] = 65 cols
# score-chunk structure: m-tiles grouped 3 at a time (exp batch = 3 banks);
# the leftover 2-wide chunk goes LAST — it shortens each head's closing
# PV tail, which gates the denominator chain (measured better than 2-first)
CHUNKS = [list(range(a, min(a + 3, MT))) for a in range(0, MT, 3)]

# ---- input layout (bf16 [128, W]) ----
SOPS, SOPT, SODS, SODT, SW = 0, 2 * N, 4 * N, 5 * N, 6 * N
WPAD = 6144              # weight block cols
SHW = SW + WPAD          # 30720
WOFF = {"wq": 0, "wk": 512, "wv": 1024, "wm": 1536, "m1": 2048,
        "m2": 4096, "d1": 5120, "d2": 5376}

_FLAY = {"bq": 0, "bm": 2, "m1g": 4, "m1be": 8, "d1g": 12, "d1be": 14,
         "d2b": 16}
WF = 18

_CACHE = {}


def _build_program(dbg=False):
    import contextlib
    import concourse.bass as bass
    import concourse.bacc as bacc
    import concourse.tile as tile
    import concourse.mybir as mybir

    FP32 = mybir.dt.float32
    BF16 = mybir.dt.bfloat16
    I32 = mybir.dt.int32
    AF = mybir.ActivationFunctionType
    ALU = mybir.AluOpType
    AX = mybir.AxisListType

    nc = bacc.Bacc(
        "TRN2",
        target_bir_lowering=False,
        debug=False,
        enable_asserts=False,
        num_devices=NC,
    )

    xin = nc.dram_tensor("xin", [128, SHW], BF16, kind="ExternalInput").ap()
    fin = nc.dram_tensor("fin", [128, WF], FP32, kind="ExternalInput").ap()
    chain = nc.dram_tensor("chain", [1, 1], FP32, kind="ExternalInput").ap()
    out_dram = nc.dram_tensor("out", [1, 1], FP32, kind="ExternalOutput").ap()

    _dbg_done = set()

    def mkdbg(dma_fn, name, src_ap, shape, dt):
        if not dbg or name in _dbg_done:
            return
        _dbg_done.add(name)
        d = nc.dram_tensor(name, list(shape), dt, kind="ExternalOutput").ap()
        dma_fn(d[tuple(slice(None) for _ in shape)], src_ap)

    with tile.TileContext(nc) as tc:
        st = contextlib.ExitStack()
        PA = st.enter_context(tc.tile_pool(name="persistA", bufs=1))
        PB = st.enter_context(tc.tile_pool(name="persistB", bufs=1))
        Ppr = st.enter_context(tc.tile_pool(name="probs", bufs=4))
        Prd = st.enter_context(tc.tile_pool(name="rdpool", bufs=2))
        Psc = st.enter_context(
            tc.tile_pool(name="psum_sc", bufs=2, space=bass.MemorySpace.PSUM))
        Pout = st.enter_context(
            tc.tile_pool(name="psum_out", bufs=2, space=bass.MemorySpace.PSUM))

        def pa(name, shape, dt, tag=None):
            return PA.tile(shape, dt, name=name, tag=tag or name)

        def pb(name, shape, dt, tag=None):
            return PB.tile(shape, dt, name=name, tag=tag or name)

        # --- persistent sbuf tensors (16 KB/partition slabs alias-chained) ---
        xs_bf = pa("xs_bf", [128, 2, N], BF16)     # raw p_srcT
        xt_bf = pa("xt_bf", [128, 2, N], BF16)     # raw p_tarT
        ddr = pa("ddr", [128, 2, N], BF16)         # raw dis_srcT / dis_tarT
        Qb = pa("Qb", [128, 2, N], BF16)
        An = pa("An", [64, 2, H, 512], BF16)       # attn out, 2 qg-slots deep
        d_bf = pa("d_bf", [128, 2, N], BF16)       # round-1 delta
        ds_full = pa("ds_full", [128, 2, N], BF16)  # mlp_dis out s
        x1fl = pb("x1f", [128, 2, 8, 512], BF16)   # mlp_dis hidden, s graph
        x1ft = pa("x1ft", [128, 2, 8, 512], BF16)  # mlp_dis hidden, t graph
        # alias chains (same tag = same memory; dependency tracking is
        # whole-tile, so every alias switch is ordered by full phases):
        #   ddr slot: raw dis (read out by d1 during r1a) -> dt2 (mlp out t)
        #   x1f slot: hidden s -> db2 (r1b out, gated in place = V of r2b)
        #             -> dp1 (r2b out)
        #   d_bf slot: r1a out -> gated in place (V of r2a) -> dp0 (r2a out)
        dt2 = pa("dt2", [128, 2, N], BF16, tag="ddr")
        db2 = pb("db2", [128, 2, N], BF16, tag="x1f")
        dp1_bf = pb("dp1_bf", [128, 2, N], BF16, tag="x1f")  # r2b out
        dp0_bf = pa("dp0_bf", [128, 2, N], BF16, tag="d_bf")  # r2a out
        VT = pb("VT", [128, MT, H * HB], BF16)
        Kb = pb("Kb", [128, 2, 8, 512], BF16)
        ones = pa("ones", [128, 64], FP32)
        wq_s = pa("wq_s", [128, 2, 256], BF16)
        wk_s = pa("wk_s", [128, 2, 256], BF16)
        wv_s = pa("wv_s", [128, 2, 256], BF16)
        wm_r = pa("wm_r", [64, 4, 256], BF16)
        m1_s = pa("m1_s", [128, 4, 512], BF16)
        d1_s = pa("d1_s", [128, 256], BF16)
        d2_s = pa("d2_s", [128, 2, 256], BF16)
        wq_c = {g: pa(f"wq_c{g}", [128, 2, 256], BF16) for g in "st"}
        wk_c = {g: pa(f"wk_c{g}", [128, 2, 256], BF16) for g in "st"}
        wv_c = {g: pa(f"wv_c{g}", [128, 2, 256], BF16) for g in "st"}
        m1_c = {g: pa(f"m1_c{g}", [128, 2, 512], BF16) for g in "st"}
        d1_c = {g: pa(f"d1_c{g}", [128, 256], BF16) for g in "st"}
        bq_c = {g: pa(f"bq_c{g}", [128, 2, 1], FP32) for g in "st"}
        bm_c = {g: pa(f"bm_c{g}", [128, 2, 1], FP32) for g in "st"}
        m_bf = {g: pa(f"m_bf{g}", [128, 2, 1], BF16) for g in "st"}
        s1b = {g: pa(f"s1b_{g}", [128, 2, 1], BF16) for g in "st"}
        s1h = {g: pa(f"s1h_{g}", [64, 4, 1], BF16) for g in "st"}
        bq_s = pa("bq_s", [128, 2, 1], FP32)
        bm_s = pa("bm_s", [128, 2, 1], FP32)
        m1g_s = pa("m1g_s", [128, 4, 1], FP32)
        m1be_s = pa("m1be_s", [128, 4, 1], FP32)
        d1g_s = pa("d1g_s", [128, 2, 1], FP32)
        d1be_s = pa("d1be_s", [128, 2, 1], FP32)
        d2b_s = pa("d2b_s", [128, 2, 1], FP32)
        bns = pa("bns", [128, 8, 6], FP32)
        bns2 = pa("bns2", [128, 2, 2, 8, 6], FP32)
        bnm = pa("bnm", [128, 4, QG, 6], FP32)
        rsum = pa("rsum", [128, 4, QG], FP32)
        tsum = pa("tsum", [128, 4, 2], FP32)
        sdif = pa("sdif", [128, 4, 1], BF16)
        dlt = pa("dlt", [128, 2, 1], FP32)
        dsq = pa("dsq", [128, 2, 1], FP32)
        res = pa("res", [1, 1], FP32)
        chn = pa("chn", [1, 1], FP32)
        sv = pa("sv", [128, 160], FP32)

        dsr = ddr[:, 0, :]
        dtr = ddr[:, 1, :]

        _svc = [0]

        def scol(n=1):
            b = _svc[0]
            _svc[0] += n
            assert _svc[0] <= 160
            return [sv[:, b + i:b + i + 1] for i in range(n)]

        dma = nc.sync.dma_start

        # ---------------- input loads ----------------
        # xt + weights first: they gate the t-stats -> t-folds -> K/V
        # projection startup chain; xs/dis can land later
        dma(xt_bf[:, :, :],
            xin[:, SOPT:SOPT + 2 * N].rearrange("p (g f) -> p g f", g=2))

        def wld(dst, nm, gcols, ngroups, flat=False):
            base = SW + WOFF[nm]
            for g in range(ngroups):
                src = xin[:, base + g * gcols:base + (g + 1) * gcols]
                if flat:
                    dma(dst[:, :], src)
                else:
                    dma(dst[:, g, :], src)

        # wk first: the K projection is the first consumer at startup
        wld(wk_s, "wk", 256, 2)
        wld(wv_s, "wv", 256, 2)
        wld(wq_s, "wq", 256, 2)
        wld(m1_s, "m1", 512, 4)
        wld(d1_s, "d1", 256, 1, flat=True)
        wld(d2_s, "d2", 256, 2)
        for h in range(H):
            co = SW + WOFF["wm"] + (h // 2) * 256
            dma(wm_r[:, h, :],
                xin[(h % 2) * 64:(h % 2) * 64 + 64, co:co + 256])

        def fld(dst, nm, g):
            o = _FLAY[nm]
            dma(dst[:, :, :],
                fin[:, o:o + g].rearrange("p (g c) -> p g c", c=1))

        fld(bq_s, "bq", 2)
        fld(bm_s, "bm", 2)
        fld(m1g_s, "m1g", 4)
        fld(m1be_s, "m1be", 4)
        fld(d1g_s, "d1g", 2)
        fld(d1be_s, "d1be", 2)
        fld(d2b_s, "d2b", 2)
        dma(chn[:, :], chain[:, :])
        dma(xs_bf[:, :, :],
            xin[:, SOPS:SOPS + 2 * N].rearrange("p (g f) -> p g f", g=2))
        dma(ddr[:, :, :],
            xin[:, SODS:SODS + 2 * N].rearrange("p (g f) -> p g f", g=2))
        nc.gpsimd.memset(ones[:, :], 1.0)

        def inv_std(var_ap, eps):
            # rsqrt on DVE (magic seed + 3 Newton steps): ScalarE Sqrt would
            # thrash the ACT table set against Exp
            v, y, a = scol(3)
            nc.vector.tensor_scalar_add(v, var_ap, float(eps))
            nc.vector.tensor_scalar(y.bitcast(I32), v.bitcast(I32), 1, None,
                                    op0=ALU.logical_shift_right)
            nc.vector.tensor_scalar(y.bitcast(I32), y.bitcast(I32), -1,
                                    0x5F3759DF, op0=ALU.mult, op1=ALU.add)
            for _ in range(3):
                nc.vector.tensor_mul(a, y, y)
                nc.vector.tensor_mul(a, a, v)
                nc.vector.tensor_scalar(a, a, -0.5, 1.5,
                                        op0=ALU.mult, op1=ALU.add)
                nc.vector.tensor_mul(y, y, a)
            return y

        # ---- standardize stats (local, bn_stats over full N). Only the
        # t-graph stats run up front; the rest are deferred into r1a's
        # pre-Q hook so the PE starts projecting ~60 us earlier ----
        stat_mu, stat_var = {}, {}

        def stat_one(key, src):
            srcr = src.rearrange("p (c f) -> p c f", f=512)
            for c in range(8):
                nc.vector.bn_stats(bns[:, c, :], srcr[:, c, :])
            ag2 = pa(f"ag_{key}", [128, 2], FP32)
            nc.vector.bn_aggr(ag2[:, :], bns[:, :, :])
            stat_mu[key], stat_var[key] = ag2[:, 0:1], ag2[:, 1:2]

        stat_one("t0", xt_bf[:, 0, :])
        stat_one("t1", xt_bf[:, 1, :])

        # ---- p stats + folded weight prep. The t-graph folds run first
        # (round 1a needs them for its K/V projections); the s-graph folds
        # are emitted inside r1a between the V and Q projections so the PE
        # does not stall on the DVE fold chain at startup ----
        invp, nbp = {}, {}

        def fold_weights(g):
            for cg in range(2):
                mu = stat_mu[f"{g}{cg}"]
                inv = inv_std(stat_var[f"{g}{cg}"], 0.0)
                (nb,) = scol(1)
                nc.vector.tensor_mul(nb, mu, inv)
                nc.vector.tensor_scalar_mul(nb, nb, -1.0)
                invp[(g, cg)], nbp[(g, cg)] = inv, nb
                nc.vector.tensor_copy(m_bf[g][:, cg, :], mu)
                nc.vector.tensor_scalar_mul(wq_c[g][:, cg, :], wq_s[:, cg, :],
                                            inv)
                nc.vector.tensor_scalar_mul(wk_c[g][:, cg, :], wk_s[:, cg, :],
                                            inv)
                nc.vector.tensor_scalar_mul(wv_c[g][:, cg, :], wv_s[:, cg, :],
                                            inv)
                nc.vector.tensor_scalar_mul(m1_c[g][:, cg, :], m1_s[:, cg, :],
                                            inv)

        def fold_biases(g):
            # bias corrections: bq' = bq - Wq'^T m ; bm' = bm - Wm (Wv'^T m)
            qsh = Psc.tile([128, 3, 512], FP32, tag="sc", name=f"qsh_{g}")
            for og in range(2):
                for cg in range(2):
                    nc.tensor.matmul(qsh[:, og, 0:1],
                                     wq_c[g][:, cg, og * 128:(og + 1) * 128],
                                     m_bf[g][:, cg, :],
                                     start=(cg == 0), stop=(cg == 1))
            for og in range(2):
                nc.vector.tensor_sub(bq_c[g][:, og, :], bq_s[:, og, :],
                                     qsh[:, og, 0:1])
            vsh = Psc.tile([128, 3, 512], FP32, tag="sc", name=f"vsh_{g}")
            for og in range(2):
                for cg in range(2):
                    nc.tensor.matmul(vsh[:, og, 0:1],
                                     wv_c[g][:, cg, og * 128:(og + 1) * 128],
                                     m_bf[g][:, cg, :],
                                     start=(cg == 0), stop=(cg == 1))
            for og in range(2):
                nc.vector.tensor_copy(s1b[g][:, og, :], vsh[:, og, 0:1])
            for h in range(H):
                dma(s1h[g][:, h, :],
                    s1b[g][(h % 2) * 64:(h % 2) * 64 + 64, h // 2, :])
            msh = Psc.tile([128, 3, 512], FP32, tag="sc", name=f"msh_{g}")
            for og in range(2):
                for h in range(H):
                    nc.tensor.matmul(
                        msh[:, og, 0:1], wm_r[:, h, og * 128:(og + 1) * 128],
                        s1h[g][:, h, :],
                        start=(h == 0), stop=(h == 3))
            for og in range(2):
                nc.vector.tensor_sub(bm_c[g][:, og, :], bm_s[:, og, :],
                                     msh[:, og, 0:1])

        # only the t-graph WEIGHT folds gate r1a's K/V projections; its
        # bias matvecs (needed first at r1a's merges / in r1b) are deferred
        # into r1a's pre-Q hook along with the whole s-graph fold
        fold_weights("t")

        def deferred_stats():
            # emitted between r1a's V and Q projections: PE is busy on K/V
            # while DVE runs these chains; Q proj then waits only on the
            # s-graph folds
            fold_biases("t")
            stat_one("s0", xs_bf[:, 0, :])
            stat_one("s1", xs_bf[:, 1, :])
            fold_weights("s")
            fold_biases("s")

        def deferred_dis_stats():
            # emitted after r1a's Q projection; only needed by the first d1
            # piece, which rides ~150 us into r1a's streaming
            stat_one("ds", dsr)
            stat_one("dt", dtr)
            for key, g in (("ds", "s"), ("dt", "t")):
                inv = inv_std(stat_var[key], 0.0)
                nc.vector.tensor_scalar_mul(d1_c[g][:, :], d1_s[:, :], inv)

        # ---------------- attention ----------------
        for h in range(H):
            nc.gpsimd.memset(VT[:, :, h * HB + HD], 1.0)

        def attention(tag, q_src, k_src, v_src, out_bf, wq, bq, wk, wv, bm,
                      extra=None, pre_q=None, post_proj=None):
            # K projection first (full N, no bias: softmax-invariant).
            # Evacuations alternate DVE/ACT so neither serializes the
            # transition between attentions.
            k_r = k_src.rearrange("p g (c f) -> p g c f", f=512)
            for og in range(2):
                for c in range(8):
                    kp = Psc.tile([128, 3, 512], FP32, tag="sc",
                                  name=f"kp_{tag}")
                    for cg in range(2):
                        nc.tensor.matmul(kp[:, 0, :],
                                         wk[:, cg, og * 128:(og + 1) * 128],
                                         k_r[:, cg, c, :],
                                         start=(cg == 0), stop=(cg == 1))
                    if c % 2 == 0:
                        nc.vector.tensor_copy(Kb[:, og, c, :], kp[:, 0, :])
                    else:
                        nc.scalar.activation(Kb[:, og, c, :], kp[:, 0, :],
                                             AF.Identity)
            # V^T projection (keys on partitions), bias folded into bm
            for m in range(MT):
                vp = Psc.tile([128, 3, 512], FP32, tag="sc", name=f"vp_{tag}")
                for cg in range(2):
                    nc.tensor.matmul(vp[:, 0, 0:256],
                                     v_src[:, cg, m * 128:(m + 1) * 128],
                                     wv[:, cg, :],
                                     start=(cg == 0), stop=(cg == 1))
                vtd = VT[:, m, :].rearrange("p (h c) -> p h c", c=HB)[:, :, 0:HD]
                vts = vp[:, 0, 0:256].rearrange("p (h c) -> p h c", c=HD)
                if m % 2 == 0:
                    nc.vector.tensor_copy(vtd, vts)
                else:
                    nc.scalar.activation(vtd, vts, AF.Identity)
            # Q projection (+bias) last
            if pre_q is not None:
                pre_q()
            for og in range(2):
                for qp_i in range(QG // 2):
                    qp = Psc.tile([128, 3, 512], FP32, tag="sc",
                                  name=f"qp_{tag}")
                    for j in range(2):
                        qg = qp_i * 2 + j
                        for cg in range(2):
                            nc.tensor.matmul(
                                qp[:, j, :],
                                wq[:, cg, og * 128:(og + 1) * 128],
                                q_src[:, cg, qg * 512:(qg + 1) * 512],
                                start=(cg == 0), stop=(cg == 1))
                    nc.vector.tensor_scalar_add(
                        Qb[:, og, qp_i * 1024:(qp_i + 1) * 1024],
                        qp[:, 0:2, :].rearrange("p a b -> p (a b)"),
                        bq[:, og, :])
            if post_proj is not None:
                post_proj()

            # streaming attention: one flat software pipeline over every
            # (qg, h, chunk) step so the ScalarE exp stream never breaks at
            # head boundaries; PV runs one chunk behind exp; evac/denominator
            # fire as each head's last PV lands; normalize rolls one head
            # behind; merge(qg) right after its 4th head normalizes
            def merge_qg(qg):
                mg = Psc.tile([128, 3, 512], FP32, tag="sc",
                              name=f"mg_{tag}")
                for og in range(2):
                    for h in range(H):
                        nc.tensor.matmul(
                            mg[:, og, :],
                            wm_r[:, h, og * 128:(og + 1) * 128],
                            An[:, qg % 2, h, :],
                            start=(h == 0), stop=(h == 3))
                for og in range(2):
                    osl = out_bf[:, og, qg * 512:(qg + 1) * 512]
                    if bm is not None:
                        nc.vector.tensor_scalar_add(osl, mg[:, og, :],
                                                    bm[:, og, :])
                    else:
                        nc.vector.tensor_copy(osl, mg[:, og, :])

            recs = {}
            prev_hq = None
            ops = {}
            pending = []

            def norm_step(qg, h, dn):
                # broadcast 1/denom from partition 0 to 0..63 inside the
                # (otherwise unused) rows of the dn tile, on idle GpSimd,
                # then evacuate+normalize the numerator straight out of
                # PSUM in a single DVE multiply
                op = ops.pop((qg, h))
                nc.gpsimd.partition_broadcast(dn[0:64, :], dn[0:1, :])
                nc.vector.tensor_mul(An[:, qg % 2, h, :],
                                     op[0:64, :], dn[0:64, :])
                if h == H - 1:
                    merge_qg(qg)
                    if extra is not None:
                        # riding work is queued and drained one piece per
                        # head completion so it never bursts the psum ring
                        pending.extend(extra(qg))
                if pending:
                    pending.pop(0)()

            def pv_emit(qg, h, chunk, pr):
                op = ops[(qg, h)]
                for j, m in enumerate(chunk):
                    nc.tensor.matmul(op[:, :], VT[:, m, h * HB:(h + 1) * HB],
                                     pr[:, j, :], start=(m == 0),
                                     stop=(m == MT - 1))

            def head_done(qg, h):
                # denominator chain only: psum row 64 -> sbuf row 64 (DVE)
                # -> partition 0 (DMA) -> fast reciprocal. The numerator
                # stays in PSUM until the rolled normalize (one head later)
                # evacuates it fused with the 1/denom multiply.
                nonlocal prev_hq
                op = ops[(qg, h)]
                dn = Ppr.tile([65, 512], FP32, tag="dnm", bufs=2,
                              name=f"dnm_{tag}{h}_{qg}")
                nc.vector.tensor_copy(dn[64:65, :], op[64:65, :])
                dma(dn[0:1, :], dn[64:65, :])
                nc.vector.reciprocal_approx_fast(dn[0:1, :], dn[0:1, :])
                recs[(qg, h)] = dn
                if prev_hq is not None:
                    norm_step(prev_hq[0], prev_hq[1], recs[prev_hq])
                prev_hq = (qg, h)

            prev = None
            for qg in range(QG):
                for h in range(H):
                    hg, hp = h // 2, (h % 2) * 64
                    qsl = Qb[hp:hp + 64, hg, qg * 512:(qg + 1) * 512]
                    ops[(qg, h)] = Pout.tile([65, 512], FP32, tag="out",
                                             name=f"op_{tag}{h}_{qg}")
                    for chunk in CHUNKS:
                        w = len(chunk)
                        sc = Psc.tile([128, 3, 512], FP32, tag="sc",
                                      name=f"sc_{tag}")
                        for j, m in enumerate(chunk):
                            c, f0 = divmod(m * 128, 512)
                            nc.tensor.matmul(sc[:, j, :],
                                             Kb[hp:hp + 64, hg, c, f0:f0 + 128],
                                             qsl,
                                             start=True, stop=True)
                        pr = Ppr.tile([128, 3, 512], BF16, tag="pr",
                                      name=f"pr_{tag}", bufs=2)
                        nc.scalar.activation(pr[:, 0:w, :], sc[:, 0:w, :],
                                             AF.Exp, scale=0.125)
                        if prev is not None:
                            pv_emit(*prev)
                            if prev[2][-1] == MT - 1:
                                head_done(prev[0], prev[1])
                        prev = (qg, h, chunk, pr)
            pv_emit(*prev)
            head_done(prev[0], prev[1])
            norm_step(prev_hq[0], prev_hq[1], recs[prev_hq])
            while pending:
                pending.pop(0)()

        def gate_chunk(buf, oraw, gkey, qg, act=True):
            # in-place: buf chunk *= std(raw chunk); std = inv*raw + nb.
            # act=False keeps the std computation off ScalarE (used when
            # riding inside an attention stream, where exp owns ScalarE)
            sl = slice(qg * 512, (qg + 1) * 512)
            for cg in range(2):
                xsd = Prd.tile([128, 2, 512], BF16, tag="rdc",
                               name=f"xsd_{gkey}")
                if act:
                    nc.scalar.activation(xsd[:, 0, :], oraw[:, cg, sl],
                                         AF.Identity, bias=nbp[(gkey, cg)],
                                         scale=invp[(gkey, cg)])
                else:
                    nc.vector.tensor_scalar(xsd[:, 0, :], oraw[:, cg, sl],
                                            invp[(gkey, cg)], nbp[(gkey, cg)],
                                            op0=ALU.mult, op1=ALU.add)
                nc.vector.tensor_mul(buf[:, cg, sl], buf[:, cg, sl],
                                     xsd[:, 0, :])

        # ---- mlp_dis: d1 chunks ride r1a's per-qg callback; hidden for the
        # t graph overwrites the raw dis slot chunk-for-chunk ----
        hid = {"s": x1fl, "t": x1ft}
        dab = {}

        def dis_d1_chunk(qg):
            def piece(gi, g):
                def run():
                    for og in range(2):
                        mp = Psc.tile([128, 3, 512], FP32, tag="sc",
                                      name="mp_d1")
                        nc.tensor.matmul(
                            mp[:, 0, :],
                            d1_c[g][:, og * 128:(og + 1) * 128],
                            ddr[:, gi, qg * 512:(qg + 1) * 512],
                            start=True, stop=True)
                        nc.vector.tensor_copy(hid[g][:, og, qg, :],
                                              mp[:, 0, :])
                        nc.vector.bn_stats(bns2[:, gi, og, qg, :],
                                           hid[g][:, og, qg, :])
                return run
            return [piece(0, "s"), piece(1, "t")]

        def dis_finalize():
            for gi, g in ((0, "s"), (1, "t")):
                ab = []
                for og in range(2):
                    ag2 = pa(f"agx_{g}{og}", [128, 2], FP32)
                    nc.vector.bn_aggr(ag2[:, :], bns2[:, gi, og, :, :])
                    inv = inv_std(ag2[:, 1:2], EPS)
                    a_, b_ = scol(2)
                    nc.vector.tensor_mul(a_, d1g_s[:, og, :], inv)
                    nc.vector.tensor_mul(b_, ag2[:, 0:1], a_)
                    nc.vector.tensor_scalar_mul(b_, b_, -1.0)
                    nc.vector.tensor_add(b_, b_, d1be_s[:, og, :])
                    ab.append((a_, b_))
                dab[g] = ab

        def dis_d2_phase():
            for c in range(8):
                for g, dfull in (("s", ds_full), ("t", dt2)):
                    ab = dab[g]
                    dfr = dfull.rearrange("p g (c f) -> p g c f", f=512)
                    rdc = Prd.tile([128, 2, 512], BF16, tag="rdc",
                                   name=f"rdc_{g}{c}")
                    for cg in range(2):
                        nc.scalar.activation(rdc[:, cg, :],
                                             hid[g][:, cg, c, :],
                                             AF.Relu, bias=ab[cg][1],
                                             scale=ab[cg][0])
                    for og in range(2):
                        mp = Psc.tile([128, 3, 512], FP32, tag="sc",
                                      name="mp_d2")
                        for cg in range(2):
                            nc.tensor.matmul(
                                mp[:, 0, :],
                                d2_s[:, cg, og * 128:(og + 1) * 128],
                                rdc[:, cg, :],
                                start=(cg == 0), stop=(cg == 1))
                        nc.vector.tensor_scalar_add(dfr[:, og, c, :],
                                                    mp[:, 0, :],
                                                    d2b_s[:, og, :])

        # ---- m1 (pass1: bn_stats on psum; pass2: relu+sum); chunks ride
        # the round-2 attentions' per-qg callbacks ----
        mab = {}
        m1_src = {"s": (xs_bf, dp0_bf), "t": (xt_bf, dp1_bf)}

        def m1_ogs(qg, gkey, pass2, ogs, relu_act=True):
            oraw, dp = m1_src[gkey]
            sl = slice(qg * 512, (qg + 1) * 512)
            rhs = [oraw[:, 0, sl], oraw[:, 1, sl],
                   dp[:, 0, sl], dp[:, 1, sl]]
            for og in ogs:
                mp = Psc.tile([128, 3, 512], FP32, tag="sc",
                              name=f"mp_m1{gkey}{int(pass2)}")
                for cg in range(4):
                    lhs = (m1_c[gkey][:, cg, og * 128:(og + 1) * 128]
                           if cg < 2 else
                           m1_s[:, cg, og * 128:(og + 1) * 128])
                    nc.tensor.matmul(mp[:, 0, :], lhs, rhs[cg],
                                     start=(cg == 0), stop=(cg == 3))
                if not pass2:
                    nc.vector.bn_stats(bnm[:, og, qg, :], mp[:, 0, :])
                else:
                    ab = mab[gkey]
                    rr = Prd.tile([128, 2, 512], BF16, tag="rdc",
                                  name=f"rr_{gkey}")
                    r_act = (og % 2 == 0) if relu_act == "alt" else relu_act
                    if r_act:
                        nc.scalar.activation(rr[:, 0, :], mp[:, 0, :],
                                             AF.Relu, bias=ab[og][1],
                                             scale=ab[og][0])
                    else:
                        nc.vector.tensor_scalar(rr[:, 0, :], mp[:, 0, :],
                                                ab[og][0], ab[og][1],
                                                op0=ALU.mult, op1=ALU.add)
                        nc.vector.tensor_scalar(rr[:, 0, :], rr[:, 0, :],
                                                0.0, None, op0=ALU.max)
                    nc.vector.reduce_sum(rsum[:, og, qg:qg + 1],
                                         rr[:, 0:1, :], axis=AX.X)

        def m1_pieces(qg, gkey, pass2, relu_act=True):
            return [
                (lambda ogs=ogs: m1_ogs(qg, gkey, pass2, ogs, relu_act))
                for ogs in ((0, 1), (2, 3))]

        def m1_finalize(gkey):
            for og in range(4):
                ag2 = pa(f"agm_{gkey}{og}", [128, 2], FP32)
                nc.vector.bn_aggr(ag2[:, :], bnm[:, og, :, :])
                inv = inv_std(ag2[:, 1:2], EPS)
                a_, b_ = scol(2)
                nc.vector.tensor_mul(a_, m1g_s[:, og, :], inv)
                nc.vector.tensor_mul(b_, ag2[:, 0:1], a_)
                nc.vector.tensor_scalar_mul(b_, b_, -1.0)
                nc.vector.tensor_add(b_, b_, m1be_s[:, og, :])
                mab.setdefault(gkey, []).append((a_, b_))

        def m1_sum(scol_i):
            for og in range(4):
                nc.vector.reduce_sum(tsum[:, og, scol_i:scol_i + 1],
                                     rsum[:, og:og + 1, :], axis=AX.X)

        # ---------------- schedule ----------------
        attention("r1a", xs_bf, xt_bf, xt_bf, d_bf,
                  wq_c["s"], bq_c["s"], wk_c["t"], wv_c["t"], bm_c["t"],
                  extra=dis_d1_chunk, pre_q=deferred_stats,
                  post_proj=deferred_dis_stats)
        mkdbg(dma, "dbg_dbf", d_bf[:, :, :], (128, 2, N), BF16)

        def r1b_post_proj():
            # hidden under r1b's projection phase: gate0 in place
            # (d_bf *= std(xt), becomes V of r2a), dis BN finalize, and the
            # relu+d2 half of mlp_dis
            for qg in range(QG):
                gate_chunk(d_bf, xt_bf, "t", qg)
            dis_finalize()
            dis_d2_phase()

        # r1b writes into the x1f slot (freed by dis_d2) and gates in place
        attention("r1b", xt_bf, xs_bf, xs_bf, db2,
                  wq_c["t"], bq_c["t"], wk_c["s"], wv_c["s"], bm_c["s"],
                  extra=lambda qg: [lambda: gate_chunk(db2, xs_bf, "s", qg,
                                                       act=False)],
                  post_proj=r1b_post_proj)
        mkdbg(dma, "dbg_dsfull", ds_full[:, :, :], (128, 2, N), BF16)

        attention("r2a", ds_full, dt2, d_bf, dp0_bf,
                  wq_s, bq_s, wk_s, wv_s, None,
                  extra=lambda qg: m1_pieces(qg, "s", False))
        mkdbg(dma, "dbg_dp0", dp0_bf[:, :, :], (128, 2, N), BF16)

        attention("r2b", dt2, ds_full, db2, dp1_bf,
                  wq_s, bq_s, wk_s, wv_s, None,
                  extra=lambda qg: (m1_pieces(qg, "s", True, relu_act=False)
                                    + m1_pieces(qg, "t", False)),
                  post_proj=lambda: m1_finalize("s"))
        mkdbg(dma, "dbg_dp1", dp1_bf[:, :, :], (128, 2, N), BF16)
        m1_finalize("t")
        m1_sum(0)
        for qg in range(QG):
            m1_ogs(qg, "t", True, (0, 1, 2, 3), relu_act="alt")
        m1_sum(1)

        # delta = m2 @ (S_s - S_t)/N ; result = ||delta||^2
        # m2 is loaded late into a transient tile (saves 2 KB of SBUF)
        m2_s = Prd.tile([128, 4, 256], BF16, tag="rdc", name="m2t")
        for g in range(4):
            dma(m2_s[:, g, :],
                xin[:, SW + WOFF["m2"] + g * 256:SW + WOFF["m2"] + (g + 1) * 256])
        for g in range(4):
            df, = scol(1)
            nc.vector.tensor_sub(df, tsum[:, g, 0:1], tsum[:, g, 1:2])
            nc.vector.tensor_scalar_mul(df, df, 1.0 / N)
            nc.vector.tensor_copy(sdif[:, g, :], df)
        mpv = Psc.tile([128, 3, 512], FP32, tag="sc", name="mpv")
        for og in range(2):
            for cg in range(4):
                nc.tensor.matmul(mpv[:, og, 0:1],
                                 m2_s[:, cg, og * 128:(og + 1) * 128],
                                 sdif[:, cg, :],
                                 start=(cg == 0), stop=(cg == 3))
        for og in range(2):
            nc.vector.tensor_copy(dlt[:, og, :], mpv[:, og, 0:1])
        nc.vector.tensor_mul(dsq[:, :, :], dlt[:, :, :], dlt[:, :, :])
        dot = Pout.tile([64, 512], FP32, tag="out", name="dot")
        for g in range(2):
            nc.tensor.matmul(dot[0:1, 0:1], dsq[:, g, :], ones[:, 0:1],
                             start=(g == 0), stop=(g == 1))
        nc.vector.tensor_copy(res[:, :], dot[0:1, 0:1])
        dma(out_dram[:, :], res[:, :])

        st.close()

    nc.compile()
    return nc


# head permutation: new row i = h*64+d  <- old channel d*4+h
_PERM = np.array([d * H + h for h in range(H) for d in range(HD)])


def _prep_inputs(inputs):
    bf16 = ml_dtypes.bfloat16
    f32 = np.float32

    def C(x, dt=f32):
        return np.ascontiguousarray(np.asarray(x), dtype=dt)

    p_src = C(inputs["p_src"])[0]
    p_tar = C(inputs["p_tar"])[0]
    dis_src = C(inputs["dis_src"])[0]
    dis_tar = C(inputs["dis_tar"])[0]
    aq_w = C(inputs["aq_w"]); ak_w = C(inputs["ak_w"])
    av_w = C(inputs["av_w"]); am_w = C(inputs["am_w"])

    wpad = np.zeros((128, WPAD), bf16)

    def putw(nm, arr):
        a = np.asarray(arr, bf16)
        R, Cc = a.shape
        o = WOFF[nm]
        for g in range(R // 128):
            wpad[:, o + g * Cc:o + (g + 1) * Cc] = a[g * 128:(g + 1) * 128, :]

    putw("wq", aq_w[_PERM, :].T)
    putw("wk", ak_w[_PERM, :].T)
    putw("wv", av_w[_PERM, :].T)
    putw("wm", am_w[:, _PERM].T)
    putw("m1", C(inputs["m1_w"]).T)
    putw("m2", C(inputs["m2_w"]).T)
    putw("d1", C(inputs["d1_w"]).T)
    putw("d2", C(inputs["d2_w"]).T)

    fb = np.zeros((128, WF), f32)

    def putf(nm, vec):
        v = np.asarray(vec, f32).reshape(-1)
        o = _FLAY[nm]
        for g in range(v.size // 128):
            fb[:, o + g] = v[g * 128:(g + 1) * 128]

    putf("bq", C(inputs["aq_b"])[_PERM])
    putf("bm", C(inputs["am_b"]) + am_w @ C(inputs["av_b"]))
    putf("m1g", inputs["m1_g"])
    putf("m1be", inputs["m1_be"])
    putf("d1g", inputs["d1_g"])
    putf("d1be", inputs["d1_be"])
    putf("d2b", inputs["d2_b"])

    x = np.zeros((128, SHW), bf16)
    for o, src in ((SOPS, p_src), (SOPT, p_tar)):
        t = np.asarray(src.T, bf16)
        x[:, o:o + N] = t[0:128]
        x[:, o + N:o + 2 * N] = t[128:256]
    x[:, SODS:SODS + N] = np.asarray(dis_src.T, bf16)
    x[:, SODT:SODT + N] = np.asarray(dis_tar.T, bf16)
    x[:, SW:SHW] = wpad
    return [{"xin": x, "fin": fb, "chain": np.zeros((1, 1), f32)}]


def kernel(**inputs):
    from concourse.bass_utils import run_bass_kernel_spmd

    if "nc" not in _CACHE:
        _CACHE["nc"] = _build_program()
    nc = _CACHE["nc"]
    in_maps = _prep_inputs(inputs)
    res = run_bass_kernel_spmd(nc, in_maps, core_ids=list(range(NC)))
    return np.asarray(res.results[0]["out"], np.float32).reshape(())
